# revision 2
# baseline (speedup 1.0000x reference)
"""GPTQ-style 4-bit quantized linear (x @ dequant(qweight) + bias) on 8 TRN2
cores. Column-parallel: N=11008 sharded 8 ways (1376 per core, exact — no
padding; 10 full 128-wide blocks + one 96-wide block per core).

Per-core pipeline (all on-chip after the weight DMA):
 1. Unpack: qweight nibbles -> fp8e3 SUBNORMAL bytes on DVE. A raw nibble
    byte 0x0q IS the fp8e3 subnormal value q/64, so one tensor_scalar
    (and 0x0F0F0F0F) yields even nibbles and one (lsr 4, and) yields odd
    nibbles — exact, 2 DVE ops per chunk. 5 DMA chunks sized
    [16,48,64,32,12] words: small first (early compute start), small last
    (short tail after the final HBM byte).
 2. Mains, W-stationary: per (block j, group g) one fp8 matmul
    lhsT = plane[128k, <=128n], rhs = xT_g [128k, 16t] accumulating
    PSUM_j[n, (g,t)] — all 32 groups side by side in one bank.
 3. ACT evacuates PSUM_j -> SCALL[:, j] fp16 reordered (t,g).
 4. Per block-group {0},{1-3},{4-7},{8-9},{10}: one fused DVE
    tensor_tensor SCM = SCALL * 64s (s broadcast over t, fp16 2x mode),
    then a 5-stage halving tree over g (fp16 2x tensor_tensor adds; the
    1x TensorReduce is ~2x slower) -> RED fp32.
 5. Correction: 11 small fp16 matmuls cP[n,t] = sum_g s(z+1)[g,n]*xsum[t,g]
    - bias into ONE PSUM tile; ACT stages it to SBUF (GpSimd cannot read
    PSUM) and Pool subtracts: OT = RED - CP.
 6. Two output DMAs (blocks 0-8 early, tail blocks late).

Math: out[t,n] = sum_g 64s[g,n]*(sum_{k in g} x_k q_kn / 64) - cP[n,t].

Everything is double-buffered across reps (pools bufs=2; PSUM 3 main
names x2 + correction x2 = exactly 8 banks) so the reps=R timing NEFF
pipelines; groups are issued one chunk late so the next chunk's unpack
never queues behind a stalled group chain on DVE's in-order queue.
"""

import numpy as np

import concourse.bass as bass
import concourse.tile as tile
from concourse import mybir, bacc
from concourse.alu_op_type import AluOpType
from concourse.bass_utils import run_bass_kernel_spmd

N_CORES = 8
GROUPSIZE = 128
MASK_NIB = 0x0F0F0F0F
_PSM_NAMES = 3


class Cfg:
    def __init__(self, K=4096, N_shard=1376, T=16):
        self.K = K
        self.G = K // GROUPSIZE          # 32 groups
        self.T = T
        self.N_shard = N_shard           # exact, no padding
        self.NB = 11                     # 10 full blocks + one 96-wide
        self.NW = N_shard // 8           # 172 int32 words per group row
        self.CHW = [16, 48, 64, 32, 12]  # words per unpack chunk
        self.NCH = len(self.CHW)
        self.CUMW = [sum(self.CHW[:i]) for i in range(self.NCH)]
        # blocks whose col window lies inside each chunk's col range
        self.JSCHED = {}
        nb_done = 0
        for i in range(self.NCH):
            hi = 8 * (self.CUMW[i] + self.CHW[i])
            js = []
            while nb_done < 11 and \
                    (nb_done * 128 + (96 if nb_done == 10 else 128)) <= hi:
                js.append(nb_done)
                nb_done += 1
            self.JSCHED[i] = js
        # TT/tree/ST groups: (first block, nblocks)
        self.GROUPS = [(0, 1), (1, 3), (4, 4), (8, 2), (10, 1)]
        self.BW = [128] * 10 + [96]      # block widths


FULL = Cfg()

# ---------------------------------------------------------------- host prep


def _unpack_rows(packed, rows):
    w = packed.view(np.uint32)
    out = np.empty((rows, packed.shape[1]), dtype=np.uint8)
    for b in range(8):
        out[b::8] = ((w >> np.uint32(4 * b)) & np.uint32(0xF)).astype(np.uint8)
    return out


def _unpack_cols(packed):
    w = packed.view(np.uint32)
    out = np.empty((w.shape[0], w.shape[1] * 8), dtype=np.uint8)
    for b in range(8):
        out[:, b::8] = ((w >> np.uint32(4 * b)) & np.uint32(0xF)).astype(np.uint8)
    return out


def host_prep(cfg, x, qweight, qzeros, scales, bias):
    G, T, NW, NS = cfg.G, cfg.T, cfg.NW, cfg.N_shard
    nib = _unpack_rows(np.asarray(qweight), cfg.K)        # [K, N] uint8
    znib = _unpack_cols(np.asarray(qzeros))               # [G, N] uint8
    x = np.asarray(x, dtype=np.float32)
    scales = np.asarray(scales, dtype=np.float64)
    bias = np.asarray(bias, dtype=np.float64)

    # device plane col (within group) for word m, nibble v; chunk c covers
    # contiguous cols [8*cum_c, 8*cum_c + 8*len_c)
    v = np.arange(8)
    m = np.arange(NW)
    chunk_of = np.searchsorted(np.array(cfg.CUMW + [NW]), m, side="right") - 1
    cum = np.array(cfg.CUMW)[chunk_of]
    ln = np.array(cfg.CHW)[chunk_of]
    ncol = (8 * cum[:, None] + (v[None, :] % 2) * (4 * ln[:, None])
            + 4 * (m[:, None] - cum[:, None]) + v[None, :] // 2)  # [NW, 8]

    # xt[p, g*T+t] = x[t, g*128+p], fp16
    xt = np.ascontiguousarray(
        x.reshape(T, G, 128).transpose(2, 1, 0).reshape(128, G * T)
    ).astype(np.float16)

    in_maps = []
    for c in range(N_CORES):
        sl = slice(c * NS, (c + 1) * NS)
        nib_s = nib[:, sl]
        z_s = znib[:, sl].astype(np.float64)
        s_s = scales[:, sl]
        b_s = bias[sl]

        # pack: wre word (g, m) nibble v = nib_s[g*128+p, ncol[m, v]]
        nib_g = nib_s.reshape(G, 128, NS)                 # [g, p, n]
        sel = nib_g[:, :, ncol]                           # [g, p, NW, 8]
        w = np.zeros((G, 128, NW), dtype=np.uint32)
        for vv in range(8):
            w |= sel[:, :, :, vv].astype(np.uint32) << np.uint32(4 * vv)
        wre = {}
        for cc in range(cfg.NCH):
            m0, m1 = cfg.CUMW[cc], cfg.CUMW[cc] + cfg.CHW[cc]
            wre[f"wre{cc}"] = np.ascontiguousarray(
                w.view(np.int32)[:, :, m0:m1]
                .transpose(1, 0, 2).reshape(128, G * cfg.CHW[cc])
            )

        # sT[p, j*G+g] = 64*s_s[g, 128j+p] (p < BW[j]), fp16
        sT = np.zeros((128, cfg.NB * G), dtype=np.float16)
        for j in range(cfg.NB):
            bw = cfg.BW[j]
            sT[:bw, j * G:(j + 1) * G] = \
                (64.0 * s_s[:, j * 128:j * 128 + bw]).T.astype(np.float16)

        # correction weights: szb[g, n] = s*(z+1); row G = -bias
        szb = np.zeros((G + 1, NS), dtype=np.float64)
        szb[:G] = s_s * (z_s + 1.0)
        szb[G] = -b_s
        # xsum_aug[g, t] = sum_k fp16(x)[t, k in g]; row G = 1 (bias slot)
        xs = xt.astype(np.float64).reshape(128, G, T).sum(axis=0)   # [G, T]
        xsum_aug = np.ones((G + 1, T), dtype=np.float64)
        xsum_aug[:G] = xs
        xst = np.concatenate([xt, sT], axis=1)           # [128, 512+352] f16
        szx = np.concatenate([szb, xsum_aug], axis=1).astype(np.float16)
        im = {"xst": xst, "szx": szx}
        im.update(wre)
        in_maps.append(im)
    return in_maps


def host_gather(cfg, results):
    out = np.empty((cfg.T, cfg.N_shard * N_CORES), dtype=np.float32)
    for c in range(N_CORES):
        oT = results[c]["outT"]  # [128, NB*T], (j, t) per partition
        for j in range(cfg.NB):
            bw = cfg.BW[j]
            out[:, c * cfg.N_shard + j * 128:
                   c * cfg.N_shard + j * 128 + bw] = \
                oT[:bw, j * cfg.T:(j + 1) * cfg.T].T
    return out


# ---------------------------------------------------------------- device kernel


def build_kernel(nc, cfg, reps=1):
    f32, f16, i32 = mybir.dt.float32, mybir.dt.float16, mybir.dt.int32
    u8, f8 = mybir.dt.uint8, mybir.dt.float8e3
    G, T, NB, NW, NS = cfg.G, cfg.T, cfg.NB, cfg.NW, cfg.N_shard
    NCH = cfg.NCH
    JT = NB * T                      # 176

    wre_d = [
        nc.declare_dram_parameter(f"wre{c}", [128, G * cfg.CHW[c]], i32,
                                  isOutput=False)
        for c in range(NCH)
    ]
    xst_d = nc.declare_dram_parameter("xst", [128, G * T + NB * G], f16,
                                      isOutput=False)
    szx_d = nc.declare_dram_parameter("szx", [G + 1, NS + T], f16,
                                      isOutput=False)
    out_d = nc.declare_dram_parameter("outT", [128, NB * T], f32, isOutput=True)

    with tile.TileContext(nc) as tc:
      with tc.tile_pool(name="sg", bufs=2) as sg, \
           tc.tile_pool(name="wp", bufs=2) as wp, \
           tc.tile_pool(name="psm", bufs=2, space="PSUM") as ps_m, \
           tc.tile_pool(name="psc", bufs=2, space="PSUM") as ps_c:
       for rep in range(reps):
        XST = sg.tile([128, G * T + NB * G], f16, tag="xst")
        X = XST[:, 0:G * T]
        ST = XST[:, G * T:G * T + NB * G]
        SZX = sg.tile([G + 1, NS + T], f16, tag="szx")
        SZB = SZX[:, 0:NS]
        xsum_aug = SZX[:, NS:NS + T]

        # xt first (block-0 matmuls), then sT (first TT), then szx
        nc.scalar.dma_start(out=XST[:, 0:G * T], in_=xst_d[:, 0:G * T])
        nc.scalar.dma_start(out=XST[:, G * T:], in_=xst_d[:, G * T:])
        nc.scalar.dma_start(out=SZX[:], in_=szx_d[:])

        PL = sg.tile([128, G * NS], u8, tag="pl")   # planes: [p, (g, 1376)]
        pli = PL[:].bitcast(i32).rearrange("p (g w) -> p g w", g=G)
        pl8 = PL[:].bitcast(f8)

        SCALL = sg.tile([128, NB * T * G], f16, tag="scall")   # (j, t, g)
        SCM = sg.tile([128, NB * T * G], f16, tag="scm")
        TA = sg.tile([128, 4 * T * 16], f16, tag="ta")         # tree g=16
        TB = sg.tile([128, 4 * T * 8], f16, tag="tb")          # tree g=8
        RED = sg.tile([128, JT], f32, tag="red")
        CPS = sg.tile([128, JT], f32, tag="cps")
        OT = sg.tile([128, JT], f32, tag="ot")
        CP = ps_c.tile([128, JT], f32, tag="cp", name=f"CP{rep % 2}")

        def do_block(j):
            bw = cfg.BW[j]
            PS = ps_m.tile([128, 512], f32, tag=f"m{j % _PSM_NAMES}",
                           name=f"PS{rep % 2}_{j % _PSM_NAMES}")
            for g in range(G):
                nc.tensor.matmul(
                    PS[:bw, g * T:(g + 1) * T],
                    pl8[:, g * NS + j * 128: g * NS + j * 128 + bw],
                    X[:, g * T:(g + 1) * T],
                    start=True, stop=True,
                )
            nc.scalar.copy(
                SCALL[:bw, j * T * G:(j + 1) * T * G]
                    .rearrange("p (t g) -> p g t", g=G),
                PS[:bw].rearrange("p (g t) -> p g t", g=G),
            )
            nc.tensor.matmul(
                CP[:bw, j * T:(j + 1) * T],
                SZB[:, j * 128:j * 128 + bw], xsum_aug[:, 0:T],
                start=True, stop=True,
            )

        def do_group(j0, nb):
            c0, c1 = j0 * T * G, (j0 + nb) * T * G
            s_b = (ST[:, j0 * G:(j0 + nb) * G]
                   .rearrange("p (j g) -> p j g", j=nb)
                   [:, :, None, :].broadcast_to([128, nb, T, G]))
            nc.vector.tensor_tensor(
                SCM[:, c0:c1].rearrange("p (j t g) -> p j t g", j=nb, t=T),
                SCALL[:, c0:c1].rearrange("p (j t g) -> p j t g", j=nb, t=T),
                s_b, AluOpType.mult,
            )
            jt = nb * T

            def halve(dst, src, gsz):
                sv = src.rearrange("p (jt g) -> p jt g", g=gsz)
                nc.vector.tensor_tensor(
                    dst.rearrange("p (jt g) -> p jt g", g=gsz // 2),
                    sv[:, :, 0:gsz // 2], sv[:, :, gsz // 2:gsz],
                    AluOpType.add,
                )

            halve(TA[:, 0:jt * 16], SCM[:, c0:c1], 32)
            halve(TB[:, 0:jt * 8], TA[:, 0:jt * 16], 16)
            halve(TA[:, 0:jt * 4], TB[:, 0:jt * 8], 8)
            halve(TB[:, 0:jt * 2], TA[:, 0:jt * 4], 4)
            halve(RED[:, j0 * T:(j0 + nb) * T], TB[:, 0:jt * 2], 2)
            sl = slice(j0 * T, (j0 + nb) * T)
            # GpSimd cannot read PSUM: ACT stages CP into SBUF, Pool subtracts
            nc.scalar.copy(CPS[:, sl], CP[:, sl])
            nc.gpsimd.tensor_tensor(
                OT[:, sl], RED[:, sl], CPS[:, sl], AluOpType.subtract,
            )

        gq = list(cfg.GROUPS)
        done = 0
        for c in range(NCH):
            LW = cfg.CHW[c]
            w0 = 2 * cfg.CUMW[c]
            WRE = wp.tile([128, G * 64], i32, tag="wre")
            nc.sync.dma_start(out=WRE[:, 0:G * LW], in_=wre_d[c][:])
            wv = WRE[:, 0:G * LW].rearrange("p (g m) -> p g m", g=G)
            nc.vector.tensor_scalar(
                out=pli[:, :, w0:w0 + LW], in0=wv,
                scalar1=MASK_NIB, scalar2=None,
                op0=AluOpType.bitwise_and,
            )
            nc.vector.tensor_scalar(
                out=pli[:, :, w0 + LW:w0 + 2 * LW], in0=wv,
                scalar1=4, scalar2=MASK_NIB,
                op0=AluOpType.logical_shift_right, op1=AluOpType.bitwise_and,
            )
            for j in cfg.JSCHED[c]:
                do_block(j)
            prev_done, done = done, done + len(cfg.JSCHED[c])
            # issue groups one chunk late so the next chunk's unpack is
            # never queued behind a stalled group chain (last chunk: flush)
            thresh = done if c == NCH - 1 else prev_done
            while gq and gq[0][0] + gq[0][1] <= thresh:
                do_group(*gq.pop(0))
        assert not gq, gq
        nc.scalar.dma_start(out=out_d[:, 0:144], in_=OT[:, 0:144])
        nc.scalar.dma_start(out=out_d[:, 144:], in_=OT[:, 144:])
    return nc


# ---------------------------------------------------------------- entry

_CACHE = {}


def _get_nc(cfg):
    key = (cfg.K, cfg.N_shard, cfg.T)
    if key not in _CACHE:
        nc = bacc.Bacc(num_devices=N_CORES)
        build_kernel(nc, cfg)
        nc.compile()
        _CACHE[key] = nc
    return _CACHE[key]


def kernel(x, qweight, qzeros, scales, bias):
    cfg = FULL
    in_maps = host_prep(cfg, x, qweight, qzeros, scales, bias)
    nc = _get_nc(cfg)
    res = run_bass_kernel_spmd(nc, in_maps, core_ids=list(range(N_CORES)))
    return host_gather(cfg, res.results)


# revision 3
# speedup vs baseline: 1.0324x; 1.0324x over previous
"""GPTQ-style 4-bit quantized linear (x @ dequant(qweight) + bias) on 8 TRN2
cores. Column-parallel: N=11008 sharded 8 ways (1376 per core, exact — no
padding; 10 full 128-wide blocks + one 96-wide block per core).

Per-core pipeline (all on-chip after the weight DMA):
 1. Unpack: qweight nibbles -> fp8e3 SUBNORMAL bytes on DVE. A raw nibble
    byte 0x0q IS the fp8e3 subnormal value q/64, so one tensor_scalar
    (and 0x0F0F0F0F) yields even nibbles and one (lsr 4, and) yields odd
    nibbles — exact, 2 DVE ops per chunk. 5 DMA chunks sized
    [16,48,64,32,12] words: small first (early compute start), small last
    (short tail after the final HBM byte).
 2. Mains, W-stationary: per (block j, group g) one fp8 matmul
    lhsT = plane[128k, <=128n], rhs = xT_g [128k, 16t] accumulating
    PSUM_j[n, (g,t)] — all 32 groups side by side in one bank.
 3. ACT evacuates PSUM_j -> SCALL[:, j] fp16 reordered (t,g).
 4. Per block-group {0},{1-3},{4-7},{8-9},{10}: one fused DVE
    tensor_tensor SCM = SCALL * 64s (s broadcast over t, fp16 2x mode),
    then a 5-stage halving tree over g (fp16 2x tensor_tensor adds; the
    1x TensorReduce is ~2x slower) -> RED fp32.
 5. Correction: 11 small fp16 matmuls cP[n,t] = sum_g s(z+1)[g,n]*xsum[t,g]
    - bias into ONE PSUM tile; ACT stages it to SBUF (GpSimd cannot read
    PSUM) and Pool subtracts: OT = RED - CP.
 6. Two output DMAs (blocks 0-8 early, tail blocks late).

Math: out[t,n] = sum_g 64s[g,n]*(sum_{k in g} x_k q_kn / 64) - cP[n,t].

Everything is double-buffered across reps (pools bufs=2; PSUM 3 main
names x2 + correction x2 = exactly 8 banks) so the reps=R timing NEFF
pipelines; groups are issued one chunk late so the next chunk's unpack
never queues behind a stalled group chain on DVE's in-order queue.
"""

import numpy as np

import concourse.bass as bass
import concourse.tile as tile
from concourse import mybir, bacc
from concourse.alu_op_type import AluOpType
from concourse.bass_utils import run_bass_kernel_spmd

N_CORES = 8
GROUPSIZE = 128
MASK_NIB = 0x0F0F0F0F
_PSM_NAMES = 3


class Cfg:
    def __init__(self, K=4096, N_shard=1376, T=16):
        self.K = K
        self.G = K // GROUPSIZE          # 32 groups
        self.T = T
        self.N_shard = N_shard           # exact, no padding
        self.NB = 11                     # 10 full blocks + one 96-wide
        self.NW = N_shard // 8           # 172 int32 words per group row
        self.CHW = [16, 48, 64, 32, 12]  # words per unpack chunk
        self.NCH = len(self.CHW)
        self.CUMW = [sum(self.CHW[:i]) for i in range(self.NCH)]
        # blocks whose col window lies inside each chunk's col range
        self.JSCHED = {}
        nb_done = 0
        for i in range(self.NCH):
            hi = 8 * (self.CUMW[i] + self.CHW[i])
            js = []
            while nb_done < 11 and \
                    (nb_done * 128 + (96 if nb_done == 10 else 128)) <= hi:
                js.append(nb_done)
                nb_done += 1
            self.JSCHED[i] = js
        # TT/tree/ST groups: (first block, nblocks)
        self.GROUPS = [(0, 1), (1, 3), (4, 4), (8, 2), (10, 1)]
        self.BW = [128] * 10 + [96]      # block widths


FULL = Cfg()

# ---------------------------------------------------------------- host prep


def _unpack_rows(packed, rows):
    w = packed.view(np.uint32)
    out = np.empty((rows, packed.shape[1]), dtype=np.uint8)
    for b in range(8):
        out[b::8] = ((w >> np.uint32(4 * b)) & np.uint32(0xF)).astype(np.uint8)
    return out


def _unpack_cols(packed):
    w = packed.view(np.uint32)
    out = np.empty((w.shape[0], w.shape[1] * 8), dtype=np.uint8)
    for b in range(8):
        out[:, b::8] = ((w >> np.uint32(4 * b)) & np.uint32(0xF)).astype(np.uint8)
    return out


def host_prep(cfg, x, qweight, qzeros, scales, bias):
    G, T, NW, NS = cfg.G, cfg.T, cfg.NW, cfg.N_shard
    nib = _unpack_rows(np.asarray(qweight), cfg.K)        # [K, N] uint8
    znib = _unpack_cols(np.asarray(qzeros))               # [G, N] uint8
    x = np.asarray(x, dtype=np.float32)
    scales = np.asarray(scales, dtype=np.float64)
    bias = np.asarray(bias, dtype=np.float64)

    # device plane col (within group) for word m, nibble v; chunk c covers
    # contiguous cols [8*cum_c, 8*cum_c + 8*len_c)
    v = np.arange(8)
    m = np.arange(NW)
    chunk_of = np.searchsorted(np.array(cfg.CUMW + [NW]), m, side="right") - 1
    cum = np.array(cfg.CUMW)[chunk_of]
    ln = np.array(cfg.CHW)[chunk_of]
    ncol = (8 * cum[:, None] + (v[None, :] % 2) * (4 * ln[:, None])
            + 4 * (m[:, None] - cum[:, None]) + v[None, :] // 2)  # [NW, 8]

    # xt[p, g*T+t] = x[t, g*128+p], fp16
    xt = np.ascontiguousarray(
        x.reshape(T, G, 128).transpose(2, 1, 0).reshape(128, G * T)
    ).astype(np.float16)

    in_maps = []
    for c in range(N_CORES):
        sl = slice(c * NS, (c + 1) * NS)
        nib_s = nib[:, sl]
        z_s = znib[:, sl].astype(np.float64)
        s_s = scales[:, sl]
        b_s = bias[sl]

        # pack: wre word (g, m) nibble v = nib_s[g*128+p, ncol[m, v]]
        nib_g = nib_s.reshape(G, 128, NS)                 # [g, p, n]
        sel = nib_g[:, :, ncol]                           # [g, p, NW, 8]
        w = np.zeros((G, 128, NW), dtype=np.uint32)
        for vv in range(8):
            w |= sel[:, :, :, vv].astype(np.uint32) << np.uint32(4 * vv)
        wre = {}
        for cc in range(cfg.NCH):
            m0, m1 = cfg.CUMW[cc], cfg.CUMW[cc] + cfg.CHW[cc]
            wre[f"wre{cc}"] = np.ascontiguousarray(
                w.view(np.int32)[:, :, m0:m1]
                .transpose(1, 0, 2).reshape(128, G * cfg.CHW[cc])
            )

        # sT[p, j*G+g] = 64*s_s[g, 128j+p] (p < BW[j]), fp16
        sT = np.zeros((128, cfg.NB * G), dtype=np.float16)
        for j in range(cfg.NB):
            bw = cfg.BW[j]
            sT[:bw, j * G:(j + 1) * G] = \
                (64.0 * s_s[:, j * 128:j * 128 + bw]).T.astype(np.float16)

        # correction weights: szb[g, n] = s*(z+1); row G = -bias
        szb = np.zeros((G + 1, NS), dtype=np.float64)
        szb[:G] = s_s * (z_s + 1.0)
        szb[G] = -b_s
        # xsum_aug[g, t] = sum_k fp16(x)[t, k in g]; row G = 1 (bias slot)
        xs = xt.astype(np.float64).reshape(128, G, T).sum(axis=0)   # [G, T]
        xsum_aug = np.ones((G + 1, T), dtype=np.float64)
        xsum_aug[:G] = xs
        xst = np.concatenate([xt, sT], axis=1)           # [128, 512+352] f16
        szx = np.concatenate([szb, xsum_aug], axis=1).astype(np.float16)
        im = {"xst": xst, "szx": szx}
        im.update(wre)
        in_maps.append(im)
    return in_maps


def host_gather(cfg, results):
    out = np.empty((cfg.T, cfg.N_shard * N_CORES), dtype=np.float32)
    for c in range(N_CORES):
        oT = results[c]["outT"]  # [128, NB*T], (j, t) per partition
        for j in range(cfg.NB):
            bw = cfg.BW[j]
            out[:, c * cfg.N_shard + j * 128:
                   c * cfg.N_shard + j * 128 + bw] = \
                oT[:bw, j * cfg.T:(j + 1) * cfg.T].T
    return out


# ---------------------------------------------------------------- device kernel


def build_kernel(nc, cfg, reps=1):
    f32, f16, i32 = mybir.dt.float32, mybir.dt.float16, mybir.dt.int32
    u8, f8 = mybir.dt.uint8, mybir.dt.float8e3
    G, T, NB, NW, NS = cfg.G, cfg.T, cfg.NB, cfg.NW, cfg.N_shard
    NCH = cfg.NCH
    JT = NB * T                      # 176

    wre_d = [
        nc.declare_dram_parameter(f"wre{c}", [128, G * cfg.CHW[c]], i32,
                                  isOutput=False)
        for c in range(NCH)
    ]
    xst_d = nc.declare_dram_parameter("xst", [128, G * T + NB * G], f16,
                                      isOutput=False)
    szx_d = nc.declare_dram_parameter("szx", [G + 1, NS + T], f16,
                                      isOutput=False)
    out_d = nc.declare_dram_parameter("outT", [128, NB * T], f32, isOutput=True)

    with tile.TileContext(nc) as tc:
      with tc.tile_pool(name="sg", bufs=2) as sg, \
           tc.tile_pool(name="wp", bufs=4) as wp, \
           tc.tile_pool(name="psm", bufs=2, space="PSUM") as ps_m, \
           tc.tile_pool(name="psc", bufs=2, space="PSUM") as ps_c:
       for rep in range(reps):
        XST = sg.tile([128, G * T + NB * G], f16, tag="xst")
        X = XST[:, 0:G * T]
        ST = XST[:, G * T:G * T + NB * G]
        SZX = sg.tile([G + 1, NS + T], f16, tag="szx")
        SZB = SZX[:, 0:NS]
        xsum_aug = SZX[:, NS:NS + T]

        # xt first (block-0 matmuls), then sT (first TT), then szx
        nc.scalar.dma_start(out=XST[:, 0:G * T], in_=xst_d[:, 0:G * T])
        nc.scalar.dma_start(out=XST[:, G * T:], in_=xst_d[:, G * T:])
        nc.scalar.dma_start(out=SZX[:], in_=szx_d[:])

        PL = sg.tile([128, G * NS], u8, tag="pl")   # planes: [p, (g, 1376)]
        pli = PL[:].bitcast(i32).rearrange("p (g w) -> p g w", g=G)
        pl8 = PL[:].bitcast(f8)

        SCALL = sg.tile([128, NB * T * G], f16, tag="scall")   # (j, t, g)
        SCM = sg.tile([128, NB * T * G], f16, tag="scm")
        TA = sg.tile([128, 4 * T * 16], f16, tag="ta")         # tree g=16
        TB = sg.tile([128, 4 * T * 8], f16, tag="tb")          # tree g=8
        RED = sg.tile([128, JT], f32, tag="red")
        CPS = sg.tile([128, JT], f32, tag="cps")
        OT = sg.tile([128, JT], f32, tag="ot")
        CP = ps_c.tile([128, JT], f32, tag="cp", name=f"CP{rep % 2}")

        def do_block(j):
            bw = cfg.BW[j]
            PS = ps_m.tile([128, 512], f32, tag=f"m{j % _PSM_NAMES}",
                           name=f"PS{rep % 2}_{j % _PSM_NAMES}")
            for g in range(G):
                nc.tensor.matmul(
                    PS[:bw, g * T:(g + 1) * T],
                    pl8[:, g * NS + j * 128: g * NS + j * 128 + bw],
                    X[:, g * T:(g + 1) * T],
                    start=True, stop=True,
                )
            nc.scalar.copy(
                SCALL[:bw, j * T * G:(j + 1) * T * G]
                    .rearrange("p (t g) -> p g t", g=G),
                PS[:bw].rearrange("p (g t) -> p g t", g=G),
            )
            nc.tensor.matmul(
                CP[:bw, j * T:(j + 1) * T],
                SZB[:, j * 128:j * 128 + bw], xsum_aug[:, 0:T],
                start=True, stop=True,
            )

        def do_group(j0, nb):
            c0, c1 = j0 * T * G, (j0 + nb) * T * G
            s_b = (ST[:, j0 * G:(j0 + nb) * G]
                   .rearrange("p (j g) -> p j g", j=nb)
                   [:, :, None, :].broadcast_to([128, nb, T, G]))
            nc.vector.tensor_tensor(
                SCM[:, c0:c1].rearrange("p (j t g) -> p j t g", j=nb, t=T),
                SCALL[:, c0:c1].rearrange("p (j t g) -> p j t g", j=nb, t=T),
                s_b, AluOpType.mult,
            )
            jt = nb * T

            def halve(dst, src, gsz):
                sv = src.rearrange("p (jt g) -> p jt g", g=gsz)
                nc.vector.tensor_tensor(
                    dst.rearrange("p (jt g) -> p jt g", g=gsz // 2),
                    sv[:, :, 0:gsz // 2], sv[:, :, gsz // 2:gsz],
                    AluOpType.add,
                )

            halve(TA[:, 0:jt * 16], SCM[:, c0:c1], 32)
            halve(TB[:, 0:jt * 8], TA[:, 0:jt * 16], 16)
            halve(TA[:, 0:jt * 4], TB[:, 0:jt * 8], 8)
            halve(TB[:, 0:jt * 2], TA[:, 0:jt * 4], 4)
            halve(RED[:, j0 * T:(j0 + nb) * T], TB[:, 0:jt * 2], 2)
            sl = slice(j0 * T, (j0 + nb) * T)
            # GpSimd cannot read PSUM: ACT stages CP into SBUF, Pool subtracts
            nc.scalar.copy(CPS[:, sl], CP[:, sl])
            nc.gpsimd.tensor_tensor(
                OT[:, sl], RED[:, sl], CPS[:, sl], AluOpType.subtract,
            )

        gq = list(cfg.GROUPS)
        done = 0
        for c in range(NCH):
            LW = cfg.CHW[c]
            w0 = 2 * cfg.CUMW[c]
            WRE = wp.tile([128, G * 64], i32, tag="wre")
            nc.sync.dma_start(out=WRE[:, 0:G * LW], in_=wre_d[c][:])
            wv = WRE[:, 0:G * LW].rearrange("p (g m) -> p g m", g=G)
            nc.vector.tensor_scalar(
                out=pli[:, :, w0:w0 + LW], in0=wv,
                scalar1=MASK_NIB, scalar2=None,
                op0=AluOpType.bitwise_and,
            )
            nc.vector.tensor_scalar(
                out=pli[:, :, w0 + LW:w0 + 2 * LW], in0=wv,
                scalar1=4, scalar2=MASK_NIB,
                op0=AluOpType.logical_shift_right, op1=AluOpType.bitwise_and,
            )
            for j in cfg.JSCHED[c]:
                do_block(j)
            prev_done, done = done, done + len(cfg.JSCHED[c])
            # issue groups one chunk late so the next chunk's unpack is
            # never queued behind a stalled group chain (last chunk: flush)
            thresh = done if c == NCH - 1 else prev_done
            while gq and gq[0][0] + gq[0][1] <= thresh:
                do_group(*gq.pop(0))
        assert not gq, gq
        nc.scalar.dma_start(out=out_d[:, 0:144], in_=OT[:, 0:144])
        nc.scalar.dma_start(out=out_d[:, 144:], in_=OT[:, 144:])
    return nc


# ---------------------------------------------------------------- entry

_CACHE = {}


def _get_nc(cfg):
    key = (cfg.K, cfg.N_shard, cfg.T)
    if key not in _CACHE:
        nc = bacc.Bacc(num_devices=N_CORES)
        build_kernel(nc, cfg)
        nc.compile()
        _CACHE[key] = nc
    return _CACHE[key]


def kernel(x, qweight, qzeros, scales, bias):
    cfg = FULL
    in_maps = host_prep(cfg, x, qweight, qzeros, scales, bias)
    nc = _get_nc(cfg)
    res = run_bass_kernel_spmd(nc, in_maps, core_ids=list(range(N_CORES)))
    return host_gather(cfg, res.results)
